# revision 17
# baseline (speedup 1.0000x reference)
"""Trainium2 Bass kernel for nn_EnhancedAttention (16-head attention with a
full [H,S,S] additive position bias), sharded 2-heads-per-core over 8 cores.

Strategy (per core, SPMD — same program, per-core data):
  - hiddenT (host-pretransposed, fp16) -> qT/kT/vT projections with channels on
    partitions; contraction always on the partition dim.
  - scoresT[k, q] = kT.T @ qT per head (K=64 single matmul per tile), so the
    softmax reduction over k folds into the PV matmul via a ones-column
    appended to V (sums come out as row 64 of the ctx accumulator).
  - bias: host precomputes exp(bias^T) in fp16; the device multiplies
    exp(qk) * exp(bias) on DVE in fp16 (2x mode) instead of adding before exp.
  - normalization: reciprocal of the sums row (DVE), partition-broadcast on
    gpsimd, multiplied into ctxT.
  - out projection: ctxT_norm (both heads stacked, K=128) @ WoT slice; each
    core writes an fp16 partial; the host sums the 8 partials and adds bo.

Scheduling (v2): phase A is q-chunk-outer so the output projection for a
q-chunk runs interleaved right after both heads finish it; bias tiles are
per-(h,kt,qc) [128,512] slices prefetched one full q-chunk ahead; all
PSUM->SBUF copies and the softmax-denominator reciprocal run on DVE so the
scalar engine does (almost) nothing but the exp over the score matrix, which
paces the kernel.
"""

import numpy as np

import concourse.bacc as bacc
import concourse.tile as tile
import concourse.mybir as mybir
from concourse.bass_utils import run_bass_kernel_spmd
from concourse.masks import make_identity

FP16 = mybir.dt.float16
FP32 = mybir.dt.float32
Exp = mybir.ActivationFunctionType.Exp
Ln = mybir.ActivationFunctionType.Ln
Copy = mybir.ActivationFunctionType.Copy
MULT = mybir.AluOpType.mult

P = 128
B, S, D = 2, 2048, 1024
H, HD = 16, 64
NCORES = 8
HPC = H // NCORES          # heads per core = 2
DT = D // P                # 8 d-tiles
ST = S // P                # 16 s-tiles (k tiles / out row tiles)
QC = 4                     # q chunks
QCW = S // QC              # 512
VSTRIDE = 2 * (HD + 1)     # 130: [h0 v (64) | ones | h1 v (64) | ones]

_NC_CACHE = {}


class _Bacc(bacc.Bacc):
    """Bacc that pins all activations to natural_log_exp_and_others.

    The default per-instruction table choice alternates between
    exp_and_others (for Exp) and natural_log_exp_and_others (for Ln),
    reloading the ACT table RAMs ~1.3us each time. This kernel only uses
    Copy/Exp, which all live in natural_log_exp_and_others, so empty
    the other sets (keeping list positions — act_func_set_id is the
    index) and a single table load suffices.
    """

    def insert_act_table_loads(self):
        import bass_rust as _bass_rust
        import concourse.mybir as _mybir
        from concourse.hw_specs import get_activation_tables
        has_activation = any(
            isinstance(i, _mybir.InstActivation)
            for b in self.main_func.blocks
            for i in b.instructions
        )
        if not has_activation:
            return
        tables = []
        for name, fns in get_activation_tables(self.m.arch).items():
            tables.append((name, fns if name == "natural_log_exp_and_others" else type(fns)()))
        _bass_rust.insert_act_table_loads(self, tables)


def _build_nc():
    nc = _Bacc("TRN2", target_bir_lowering=False)

    hT = nc.dram_tensor("hT", [B, P, DT, S], FP16, kind="ExternalInput")
    w3 = nc.dram_tensor("w3", [P, 3 * DT * P], FP16, kind="ExternalInput")
    eb = nc.dram_tensor("eb", [HPC, S, S], FP16, kind="ExternalInput")
    woT = nc.dram_tensor("woT", [P, D], FP16, kind="ExternalInput")
    outp = nc.dram_tensor("outp", [B, S, D], FP16, kind="ExternalOutput")

    with tile.TileContext(nc) as tc:
        # ---- persistent tiles ----
        persist = tc.alloc_tile_pool(name="persist", bufs=1)
        qT_sb = [persist.tile([P, S], FP16, tag=f"qT{b}", name=f"qT{b}") for b in range(B)]
        kT_sb = [persist.tile([P, S], FP16, tag=f"kT{b}", name=f"kT{b}") for b in range(B)]
        ctxn = [persist.tile([P, S], FP16, tag=f"ctxn{b}", name=f"ctxn{b}") for b in range(B)]
        v_all = persist.tile([P, B * ST * VSTRIDE], FP16, tag="v_all", name="v_all")
        w_sb = persist.tile([P, 3 * DT * P], FP16, tag="w_sb", name="w_sb")
        woT_sb = persist.tile([P, D], FP16, tag="woT_sb", name="woT_sb")
        ident = persist.tile([P, P], FP16, tag="ident", name="ident")
        # per-head staging: batch b on partition halves (b0 rows 0-63, b1 rows
        # 64-127) so the two batches' score matmuls occupy different PE row
        # strips and run concurrently in the array.
        qSc = [persist.tile([P, S], FP16, tag=f"qSc{h}", name=f"qSc{h}") for h in range(HPC)]
        kSc = [persist.tile([P, S], FP16, tag=f"kSc{h}", name=f"kSc{h}") for h in range(HPC)]

        make_identity(nc, ident[:])
        nc.any.memset(v_all[:], 1.0)  # ones columns survive the v copies
        nc.sync.dma_start(w_sb[:], w3[:])
        nc.sync.dma_start(woT_sb[:], woT[:])

        # bias tiles: per-(qc, h, kt) [128, 512] slices, prefetched a full
        # q-chunk ahead.  44 bufs = the 32 tiles of the active chunk plus 12
        # of the next chunk in flight.
        ebp = tc.alloc_tile_pool(name="ebp", bufs=44)
        eb_tiles = {}

        def emit_eb_dmas(qc):
            for h in range(HPC):
                for kt in range(ST):
                    t = ebp.tile([P, QCW], FP16, tag="eb", name=f"eb_{qc}_{h}_{kt}")
                    nc.sync.dma_start(
                        t[:], eb[h, kt * P:(kt + 1) * P, qc * QCW:(qc + 1) * QCW])
                    eb_tiles[(qc, h, kt)] = t

        # ---- phase P: projections + v transpose ----
        with (
            tc.tile_pool(name="hp", bufs=2) as hp,
            tc.tile_pool(name="vt", bufs=8) as vtp,
            tc.tile_pool(name="psP", bufs=5, space="PSUM") as psP,
            tc.tile_pool(name="psT", bufs=2, space="PSUM") as psT,
        ):
            h_tiles = []
            half = DT // 2
            for b in range(B):
                h_all = hp.tile([P, DT * S], FP16, tag="h", name=f"h_{b}")
                nc.sync.dma_start(h_all[:, 0:half * S], hT[b, :, 0:half])
                nc.sync.dma_start(h_all[:, half * S:], hT[b, :, half:])
                h_tiles.append(h_all)
            emit_eb_dmas(0)
            for b in range(B):
                h_all = h_tiles[b]
                h_sb = [h_all[:, dt * S:(dt + 1) * S] for dt in range(DT)]
                vT_tiles = []
                for p in range(3):
                    ps_qc = [
                        psP.tile([P, QCW], FP32, tag="pj", name=f"pj_{b}_{p}_{qc}")
                        for qc in range(QC)
                    ]
                    for dt in range(DT):
                        for qc in range(QC):
                            nc.tensor.matmul(
                                ps_qc[qc][:],
                                w_sb[:, (p * DT + dt) * P:(p * DT + dt + 1) * P],
                                h_sb[dt][:, qc * QCW:(qc + 1) * QCW],
                                start=(dt == 0), stop=(dt == DT - 1),
                            )
                    for qc in range(QC):
                        if p == 0:
                            nc.scalar.activation(
                                qT_sb[b][:, qc * QCW:(qc + 1) * QCW], ps_qc[qc][:],
                                Copy, scale=1.0 / np.sqrt(HD),
                            )
                        elif p == 1:
                            nc.vector.tensor_copy(
                                out=kT_sb[b][:, qc * QCW:(qc + 1) * QCW], in_=ps_qc[qc][:])
                        else:
                            vt = vtp.tile([P, QCW], FP16, tag="v", name=f"vt_{b}_{qc}")
                            nc.scalar.activation(vt[:], ps_qc[qc][:], Copy)
                            vT_tiles.append(vt)
                    if p == 1:
                        # stage q/k into the batch-stacked per-head layout as
                        # soon as this batch's q and k are complete
                        for h in range(HPC):
                            nc.sync.dma_start(qSc[h][b * HD:(b + 1) * HD, :],
                                              qT_sb[b][h * HD:(h + 1) * HD, :])
                            nc.sync.dma_start(kSc[h][b * HD:(b + 1) * HD, :],
                                              kT_sb[b][h * HD:(h + 1) * HD, :])
                # transpose vT [ch, s] -> v [s, ch] in 128x128 blocks (both heads at once)
                for st in range(ST):
                    tp = psT.tile([P, P], FP16, tag="tr", name=f"tr_{b}_{st}")
                    src = vT_tiles[st // 4]
                    nc.tensor.transpose(tp[:], src[:, (st % 4) * P:(st % 4 + 1) * P], ident[:])
                    base = (b * ST + st) * VSTRIDE
                    nc.vector.tensor_copy(out=v_all[:, base:base + HD], in_=tp[:, 0:HD])
                    nc.vector.tensor_copy(
                        out=v_all[:, base + HD + 1:base + 2 * HD + 1], in_=tp[:, HD:2 * HD])

        # ---- phase A: attention, q-chunk outer, both heads' kt chains
        # interleaved (doubles the PE's pool of runnable matmuls), out-proj
        # for each q-chunk interleaved right after it completes ----
        with (
            tc.tile_pool(name="pr", bufs=8) as prp,
            tc.tile_pool(name="sm", bufs=8) as smp,
            tc.tile_pool(name="op", bufs=4) as op,
            tc.tile_pool(name="psS", bufs=2, space="PSUM") as psS,
            tc.tile_pool(name="psC", bufs=4, space="PSUM") as psC,
        ):
            ocnt = 0

            def emit_norm(qc, h, b, cps):
                # softmax denom: ctx row 64 holds sum(exp(qk)*eb); divide via
                # exp(-ln(sum)) on scalar, partition-broadcast, DVE multiply.
                nln = smp.tile([1, QCW], FP32, tag="su", name=f"su_{h}_{qc}_{b}")
                nc.scalar.activation(nln[:], cps[HD:HD + 1, :], Ln)
                rcp = smp.tile([1, QCW], FP16, tag="rc", name=f"rc_{h}_{qc}_{b}")
                with nc.allow_low_precision(reason="softmax denom fp16 ok"):
                    nc.scalar.activation(rcp[:], nln[:], Exp, scale=-1.0)
                bc = smp.tile([HD, QCW], FP16, tag="bc", name=f"bcs_{h}_{qc}_{b}")
                nc.gpsimd.partition_broadcast(bc[:], rcp[:])
                nc.vector.tensor_tensor(
                    ctxn[b][h * HD:(h + 1) * HD, qc * QCW:(qc + 1) * QCW],
                    cps[0:HD, :], bc[:], MULT)

            def emit_out(qc, b, st):
                o_ps = psS.tile([P, 2 * QCW], FP32, tag="s", name=f"o_{b}_{st}")
                for ec in range(2):
                    nc.tensor.matmul(
                        o_ps[:, ec * QCW:(ec + 1) * QCW],
                        ctxn[b][:, st * P:(st + 1) * P],
                        woT_sb[:, ec * QCW:(ec + 1) * QCW],
                        start=True, stop=True,
                    )
                o_sb = op.tile([P, D], FP16, tag="ot", name=f"ot_{b}_{st}")
                nonlocal ocnt
                nc.vector.tensor_copy(out=o_sb[:], in_=o_ps[:])
                ocnt += 1
                nc.sync.dma_start(outp[b, st * P:(st + 1) * P, :], o_sb[:])

            for qc in range(QC):
                if qc + 1 < QC:
                    emit_eb_dmas(qc + 1)
                # deferred work from the previous q-chunk, woven into this
                # chunk's kt loop so the tensor queue never head-of-line
                # blocks on norm/out-proj at the chunk boundary
                pend = []
                if qc > 0:
                    pqc = qc - 1
                    for h in range(HPC):
                        for b in range(B):
                            pend.append(("n", pqc, h, b, pend_ctx[(h, b)]))
                    for b in range(B):
                        for st in range(pqc * (ST // QC), (pqc + 1) * (ST // QC)):
                            pend.append(("o", pqc, b, st, None))
                ctx_ps = {}
                for h in range(HPC):
                    for b in range(B):
                        ctx_ps[(h, b)] = psC.tile(
                            [P, QCW], FP32, tag="c", name=f"ctx_{h}_{qc}_{b}")
                for kt in range(ST):
                    prs = {}
                    for h in range(HPC):
                        s_ps = psS.tile([P, 2 * QCW], FP32, tag="s", name=f"s_{h}_{qc}_{kt}")
                        for b in range(B):
                            nc.tensor.matmul(
                                s_ps[:, b * QCW:(b + 1) * QCW],
                                kSc[h][b * HD:(b + 1) * HD, kt * P:(kt + 1) * P],
                                qSc[h][b * HD:(b + 1) * HD, qc * QCW:(qc + 1) * QCW],
                                start=True, stop=True,
                            )
                        pr = prp.tile([P, 2 * QCW], FP16, tag="p", name=f"p_{h}_{qc}_{kt}")
                        nc.scalar.activation(pr[:], s_ps[:], Exp)
                        ebs = eb_tiles.pop((qc, h, kt))
                        prv = pr.rearrange("p (b q) -> p b q", b=B)
                        nc.vector.tensor_tensor(
                            prv[:], prv[:],
                            ebs[:, None, :].to_broadcast((P, B, QCW)), MULT)
                        prs[h] = pr
                    for h in range(HPC):
                        for b in range(B):
                            vbase = (b * ST + kt) * VSTRIDE + h * (HD + 1)
                            nc.tensor.matmul(
                                ctx_ps[(h, b)][0:HD + 1, :],
                                v_all[:, vbase:vbase + HD + 1],
                                prs[h][:, b * QCW:(b + 1) * QCW],
                                start=(kt == 0), stop=(kt == ST - 1),
                            )
                    if pend:
                        item = pend.pop(0)
                        if item[0] == "n":
                            _, pqc, h, b, cps = item
                            emit_norm(pqc, h, b, cps)
                        else:
                            _, pqc, b, st, _ = item
                            emit_out(pqc, b, st)
                pend_ctx = ctx_ps
            # drain the final q-chunk's norms and out-projection
            for h in range(HPC):
                for b in range(B):
                    emit_norm(QC - 1, h, b, pend_ctx[(h, b)])
            for b in range(B):
                for st in range((QC - 1) * (ST // QC), QC * (ST // QC)):
                    emit_out(QC - 1, b, st)

        ebp.release()
        persist.release()

    nc.finalize()
    return nc


def _numpy_reference(hidden_states, attention_mask, relative_position,
                     Wq, bq, Wk, bk, Wv, bv, Wo, bo):
    Bn, Sn, Dn = hidden_states.shape
    Hn = relative_position.shape[1]
    hd = Dn // Hn
    x = hidden_states.astype(np.float64)

    def heads(t):
        return t.reshape(Bn, Sn, Hn, hd).transpose(0, 2, 1, 3)

    q = heads(x @ Wq.T.astype(np.float64) + bq)
    k = heads(x @ Wk.T.astype(np.float64) + bk)
    v = heads(x @ Wv.T.astype(np.float64) + bv)
    s = np.einsum("bhqd,bhkd->bhqk", q, k) / np.sqrt(hd)
    s = s + relative_position.astype(np.float64) + attention_mask.astype(np.float64)
    s = s - s.max(axis=-1, keepdims=True)
    p = np.exp(s)
    p /= p.sum(axis=-1, keepdims=True)
    ctx = np.einsum("bhqk,bhkd->bhqd", p, v)
    ctx = ctx.transpose(0, 2, 1, 3).reshape(Bn, Sn, Dn)
    return (ctx @ Wo.T.astype(np.float64) + bo).astype(np.float32)


def kernel(hidden_states, attention_mask, relative_position,
           Wq, bq, Wk, bk, Wv, bv, Wo, bo):
    hidden_states = np.asarray(hidden_states)
    attention_mask = np.asarray(attention_mask)
    relative_position = np.asarray(relative_position)
    Wq, bq = np.asarray(Wq), np.asarray(bq)
    Wk, bk = np.asarray(Wk), np.asarray(bk)
    Wv, bv = np.asarray(Wv), np.asarray(bv)
    Wo, bo = np.asarray(Wo), np.asarray(bo)

    # The device program folds the (always-zero) mask and qkv biases away;
    # fall back to a plain numpy path if they are ever nonzero.
    if (np.any(attention_mask) or np.any(bq) or np.any(bk) or np.any(bv)
            or hidden_states.shape != (B, S, D)):
        return _numpy_reference(hidden_states, attention_mask, relative_position,
                                Wq, bq, Wk, bk, Wv, bv, Wo, bo)

    if "nc" not in _NC_CACHE:
        _NC_CACHE["nc"] = _build_nc()
    nc = _NC_CACHE["nc"]

    hT = np.ascontiguousarray(
        hidden_states.transpose(0, 2, 1).reshape(B, DT, P, S).transpose(0, 2, 1, 3)
    ).astype(np.float16)  # [B, 128, dt, S]
    rel = relative_position[0]  # [H, S, S]

    in_maps = []
    for c in range(NCORES):
        sl = slice(c * HPC * HD, (c + 1) * HPC * HD)
        heads = rel[c * HPC:(c + 1) * HPC]  # [HPC, S, S] (q, k)
        ebT = np.exp(heads.transpose(0, 2, 1)).astype(np.float16)  # [HPC, k, q]
        w3 = np.ascontiguousarray(
            np.stack([Wq[sl].T, Wk[sl].T, Wv[sl].T])       # [3, D, 128]
            .reshape(3, DT, P, P).transpose(2, 0, 1, 3)     # [128, 3, dt, 128]
            .reshape(P, 3 * DT * P)).astype(np.float16)
        woT = np.ascontiguousarray(Wo[:, sl].T).astype(np.float16)
        in_maps.append({"hT": hT, "w3": w3, "eb": ebT, "woT": woT})

    res = run_bass_kernel_spmd(nc, in_maps, core_ids=list(range(NCORES)))
    _NC_CACHE["last_results"] = res

    out = np.zeros((B, S, D), np.float32)
    for c in range(NCORES):
        out += res.results[c]["outp"].astype(np.float32)
    out += bo.astype(np.float32)
    return out
